# revision 44
# baseline (speedup 1.0000x reference)
"""Bayesian GPLVM collapsed-ELBO kernel for Trainium2 (8 NeuronCores).

Sharding: data-parallel over n (2048 rows -> 256 per core). Each core
computes its partial psi2 = sum_n exp(log_psi2_n) over the 2080
upper-triangle (i,j) pairs, partial A = psi1^T y (64x256), and partial
KL statistics. The host sums the 8 partials and does the small m x m
linear algebra (solves / slogdets) to produce the scalar ELBO;
tr(y y^T) is an input-only reduction and is also done on the host.

Device flow per core (n_loc = 256 as two fused 128-row chunks):
  - Inputs ride the sync-engine DMA ring; the psi2 z-block zl (the
    per-partition-bytes hog) is split in three pieces with zs1/y
    slotted between, so no matmul group is ever DMA-gated.
  - Prep is fused across both chunks with chunk-strided [p, 2, w]
    views. The per-n reductions that the reference folds into scalar
    exponent rows are instead EXTENDED CONTRACTIONS: qmu^2*w2 and
    ln d2 become matmul rows paired with constant z-side rows (-1 and
    -0.5), so no reduce gates the PE transpose; psi1's h1 scalar is
    injected as a per-partition bias on its exp instead of a matmul
    row. The transpose fires right after four elementwise multiplies.
  - nprep (98 x 256 q-major, one transpose+cast per chunk): rows 0:65
    serve psi2 (zl rows [Su; -0.25 Su^2; -1; -0.5; 4*logvar - s1]),
    rows 0:97 serve psi1 (zs1x zero below row 65, then z, -0.5 z^2).
  - psi2 runs as matmul groups of [2,4,4,4,1] chunks, Exp on Scalar,
    n-sum reduces on Vector with 3-deep PSUM pipelining; the last two
    chunks are exp+accumulate singles on Scalar so the Vector reduce
    stream and the exp stream finish together. A = psi1^T y uses its
    own PSUM bank and is DMA'd out mid-kernel.
  - All activations use only {Exp, Ln}: the activation-table universe
    handed to the table-load pass is filtered (indices preserved) so
    both are served by the one table holding exp AND ln -> a single
    ACT_TABLE_LOAD that overlaps the input DMAs.
"""

import numpy as np

N, D, Q, M = 2048, 256, 16, 64
NCORES = 8
NLOC = N // NCORES          # 256

# acqin column layout
C_QM = 0        # [0:32)    qmu  (chunk0 | chunk1)
C_QLS = 32      # [32:64)   q_log_sigma
C_AL = 64      # [64:96)   alpha tiled twice
C_CONST = 96    # [96:100)  [2*logvar, 4*logvar, 0, 0]
ACQ_W = 104

_compiled = None


def _patch_act_tables():
    """Filter the activation-table universe handed to the table-load
    pass so `natural_log_exp_and_others` is the only table providing
    Exp/Ln. Table indices (act_func_set_id) are preserved, so the
    emitted loads still point at the real act_info.json entry; the
    whole kernel then needs a single ACT_TABLE_LOAD."""
    import concourse.bacc as bacc_mod
    import concourse.mybir as mybir
    from concourse.hw_specs import get_activation_tables

    def patched(arch):
        real = get_activation_tables(arch)
        target = None
        for name, funcs in real.items():
            if (mybir.ActivationFunctionType.Exp in funcs
                    and mybir.ActivationFunctionType.Ln in funcs):
                target = name
                break
        if target is None:
            return real
        strip = {mybir.ActivationFunctionType.Exp,
                 mybir.ActivationFunctionType.Ln}
        return {
            name: (set(funcs) if name == target else set(funcs) - strip)
            for name, funcs in real.items()
        }

    bacc_mod.get_activation_tables = patched


def _build_bass():
    import concourse.bacc as bacc
    import concourse.mybir as mybir
    from concourse import masks
    from concourse.tile import TileContext

    _patch_act_tables()

    f32 = mybir.dt.float32
    f32r = mybir.dt.float32r
    bf16 = mybir.dt.bfloat16
    AF = mybir.ActivationFunctionType
    OP = mybir.AluOpType
    AX = mybir.AxisListType

    nc = bacc.Bacc("TRN2", target_bir_lowering=False, num_swdge_queues=2)

    acqin_d = nc.declare_dram_parameter("acqin", [128, ACQ_W], f32, isOutput=False)
    zl_d = nc.declare_dram_parameter("zl", [65, 17 * 128], f32r, isOutput=False)
    y_d = nc.declare_dram_parameter("ybig", [128, 2 * NLOC], f32r, isOutput=False)
    zs1_d = nc.declare_dram_parameter("zs1x", [97, M], f32r, isOutput=False)
    a_o = nc.declare_dram_parameter("out_A", [M, D], f32, isOutput=True)
    st_o = nc.declare_dram_parameter("out_stats", [128, 25], f32, isOutput=True)

    with TileContext(nc) as tc:
        with (
            tc.tile_pool(name="const", bufs=1) as cpool,
            tc.tile_pool(name="big", bufs=1) as bigpool,
            tc.tile_pool(name="scr", bufs=3) as spool,
            tc.tile_pool(name="psum", bufs=3, space="PSUM") as ppool,
            tc.tile_pool(name="psums", bufs=1, space="PSUM") as ppools,
        ):
            # inputs, all on the sync (SP) ring. The 34-partition zl
            # transfer (the per-partition-bytes hog) is split in three so
            # the first psi2 groups are never DMA-gated; y rides between
            # the second and third zl pieces (needed by the A matmuls).
            acqin = cpool.tile([128, ACQ_W], f32)
            nc.sync.dma_start(out=acqin[:, :], in_=acqin_d[:, :])
            zl_sb = bigpool.tile([65, 17 * 128], f32r)
            zc1, zc2 = 2 * 128, 10 * 128
            nc.sync.dma_start(out=zl_sb[:, :zc1], in_=zl_d[:, :zc1])
            zs1_sb = cpool.tile([97, M], f32r)
            nc.sync.dma_start(out=zs1_sb[:, :], in_=zs1_d[:, :])
            nc.sync.dma_start(out=zl_sb[:, zc1:zc2], in_=zl_d[:, zc1:zc2])
            ybig = bigpool.tile([128, 2 * NLOC], f32r)
            nc.sync.dma_start(out=ybig[:, :], in_=y_d[:, :])
            nc.sync.dma_start(out=zl_sb[:, zc2:], in_=zl_d[:, zc2:])

            ident = cpool.tile([128, 128], f32)
            masks.make_identity(nc, ident[:])

            qm = acqin[:, C_QM:C_QM + 32]
            qls = acqin[:, C_QLS:C_QLS + 32]
            al2 = acqin[:, C_AL:C_AL + 32]
            c2lv = acqin[:, C_CONST:C_CONST + 1]
            zs1x = zs1_sb[:, :]

            stats = bigpool.tile([128, 25], f32)
            nprep = bigpool.tile([98, 2 * 128], f32r)
            P = bigpool.tile([128, 196], f32)
            S = bigpool.tile([128, 96], f32)      # [d1(32) | d2(32) | qsig(32)]
            L = bigpool.tile([128, 96], f32)      # ln of S
            M1 = bigpool.tile([128, 32], f32)
            R = bigpool.tile([128, 64], f32)
            scr2 = bigpool.tile([128, 64], f32)
            cols = bigpool.tile([128, 8], f32)
            qmsq = bigpool.tile([128, 32], f32)

            qsig = S[:, 64:96]
            d1 = S[:, 0:32]
            d2 = S[:, 32:64]

            # q_sigma = softplus(qls) = ln(1 + exp(qls)), both chunks
            nc.vector.tensor_mul(qmsq[:, :], qm, qm)
            nc.scalar.activation(M1[:, :], qls, AF.Exp)
            nc.scalar.activation(qsig, M1[:, :], AF.Ln, bias=1.0)
            # d1 = alpha*qsig + 1 ; d2 = 2*d1 - 1 = 2*(alpha*qsig) + 1
            nc.vector.tensor_mul(M1[:, :], qsig, al2)
            nc.vector.tensor_scalar_add(d1, M1[:, :], 1.0)
            nc.vector.tensor_scalar(out=d2, in0=M1[:, :], scalar1=2.0,
                                    scalar2=1.0, op0=OP.mult, op1=OP.add)
            # sum_q ln x = ln(prod_q x): product-reduce on DVE, tiny Ln
            # on ACT for sum2 (h1 bias); the KL ln-sigma ships as per-row
            # products that the host logs before summing across rows
            nc.vector.tensor_reduce(
                L[:, 0:2], S[:, 0:32].rearrange("p (a b) -> p a b", b=16),
                axis=AX.X, op=OP.mult)
            nc.vector.tensor_reduce(
                stats[:, 19:21], qsig.rearrange("p (a b) -> p a b", b=16),
                axis=AX.X, op=OP.mult)
            nc.scalar.activation(stats[:, 17:19], L[:, 0:2], AF.Ln)

            # Both chunks at once via chunk-strided [p, 2, w] views of P.
            # Per-chunk columns (= nprep rows after the transpose):
            # 0:16 qmu*w2, 16:32 w2, 32:48 qmu^2*w2, 48:64 ln d2,
            # 64 one, 65:81 qmu*w1, 81:97 w1, 97 pad.
            # psi2 contracts rows 0:65 against zl rows
            # [Su; -0.25*Su^2; -1; -0.5; 4*logvar - s1]; the qmu^2*w2 and
            # ln d2 rows replace the old per-n "g" scalar, so no reduce
            # gates the transpose. psi1 contracts rows 0:97 against zs1x
            # (zeros below 65, then z, -0.5 z^2), with h1 injected later
            # as a per-partition bias on the psi1 exp.
            Pv = P[:, :].rearrange("p (c s) -> p c s", c=2)
            qmv = qm.rearrange("p (c s) -> p c s", s=16)
            qmsqv = qmsq[:, :].rearrange("p (c s) -> p c s", s=16)
            alv = al2.rearrange("p (c s) -> p c s", s=16)
            nc.vector.memset(Pv[:, :, 64:65], 1.0)
            nc.vector.reciprocal(R[:, :], S[:, 0:64])
            Rv1 = R[:, 0:32].rearrange("p (c s) -> p c s", s=16)
            Rv2 = R[:, 32:64].rearrange("p (c s) -> p c s", s=16)
            nc.vector.tensor_mul(Pv[:, :, 16:32], Rv2, alv)
            nc.vector.tensor_mul(Pv[:, :, 81:97], Rv1, alv)
            nc.vector.tensor_mul(Pv[:, :, 0:16], Pv[:, :, 16:32], qmv)
            nc.vector.tensor_mul(Pv[:, :, 32:48], Pv[:, :, 0:16], qmv)
            nc.vector.tensor_mul(Pv[:, :, 65:81], Pv[:, :, 81:97], qmv)
            nc.scalar.activation(Pv[:, :, 48:64], d2, AF.Ln)

            ptp = ppools.tile([98, 256], f32, tag="ptp")
            for c in range(2):
                nc.tensor.transpose(ptp[:, 128 * c:128 * (c + 1)],
                                    P[:, 98 * c:98 * (c + 1)], ident[:, :])
                nc.vector.tensor_copy(nprep[:, 128 * c:128 * (c + 1)],
                                      ptp[:, 128 * c:128 * (c + 1)])

            # h1 = 2*logvar - 0.5*(rt1 + sum2), used as psi1 exp bias
            nc.vector.tensor_mul(
                scr2[:, 0:32].rearrange("p (c s) -> p c s", s=16),
                Pv[:, :, 81:97], qmsqv)
            nc.vector.tensor_reduce(
                cols[:, 0:2], scr2[:, 0:32].rearrange("p (a b) -> p a b", b=16),
                axis=AX.X, op=OP.add)
            nc.vector.tensor_scalar(
                out=cols[:, 4:6], in0=stats[:, 17:19], scalar1=-0.5,
                scalar2=c2lv, op0=OP.mult, op1=OP.add)
            nc.vector.scalar_tensor_tensor(
                out=cols[:, 6:8], in0=cols[:, 0:2], scalar=-0.5,
                in1=cols[:, 4:6], op0=OP.mult, op1=OP.add)

            # KL statistics (tr(y y^T) is an input-only reduction and is
            # done on the host)
            nc.vector.tensor_mul(scr2[:, 0:32], qsig, qsig)
            nc.vector.tensor_reduce(
                stats[:, 23:24], qmsq[:, :].rearrange("p (a b) -> p a b", b=32),
                axis=AX.X, op=OP.add)
            nc.vector.tensor_reduce(
                stats[:, 24:25], scr2[:, 0:32].rearrange("p (a b) -> p a b", b=32),
                axis=AX.X, op=OP.add)

            # psi2: 17 ij-chunks in groups (first group small so the ACT
            # exp stream starts as early as possible); n-sums on DVE
            GROUPS = (2, 4, 4, 4, 1)

            def psi2_group(ch0, nch):
                w = nch * NLOC
                p2 = ppool.tile([128, 4 * NLOC], f32, tag="p2")
                for j in range(nch):
                    ch = ch0 + j
                    nc.tensor.matmul(
                        p2[:, j * NLOC:(j + 1) * NLOC],
                        lhsT=zl_sb[:, ch * 128:(ch + 1) * 128],
                        rhs=nprep[0:65, :],
                        start=True, stop=True)
                scr = spool.tile([128, 4 * NLOC], f32, tag="p2scr")
                nc.scalar.activation(scr[:, :w], p2[:, :w], AF.Exp)
                nc.vector.tensor_reduce(
                    stats[:, ch0:ch0 + nch],
                    scr[:, :w].rearrange("p (a b) -> p a b", b=NLOC),
                    axis=AX.X, op=OP.add)

            psi2_group(0, GROUPS[0])

            # psi1 exponent for both chunks (borrows one rotation slot of
            # the psi2 PSUM pool, released by its exp)
            e1 = ppool.tile([128, 4 * NLOC], f32, tag="p2")
            for c in range(2):
                nc.tensor.matmul(e1[:, M * c:M * (c + 1)],
                                 lhsT=nprep[0:97, 128 * c:128 * (c + 1)],
                                 rhs=zs1x,
                                 start=True, stop=True)
            psi1c = bigpool.tile([128, 2 * M], f32r)
            for c in range(2):
                nc.scalar.activation(psi1c[:, M * c:M * (c + 1)],
                                     e1[:, M * c:M * (c + 1)], AF.Exp,
                                     bias=cols[:, 6 + c:7 + c])

            psi2_group(GROUPS[0], GROUPS[1])

            # A = psi1^T y early (own PSUM bank): its copy slots into a
            # DVE gap and the DMA completes long before the tail
            apsum = ppools.tile([M, D], f32, tag="aps")
            for c in range(2):
                nc.tensor.matmul(apsum[:, :],
                                 lhsT=psi1c[:, M * c:M * (c + 1)],
                                 rhs=ybig[:, NLOC * c:NLOC * (c + 1)],
                                 start=(c == 0), stop=(c == 1))
            a_sb = bigpool.tile([M, D], f32)
            nc.vector.tensor_copy(a_sb[:, :], apsum[:, :])
            nc.sync.dma_start(out=a_o[:, :], in_=a_sb[:, :])

            ch0 = GROUPS[0] + GROUPS[1]
            for t in range(2, len(GROUPS)):
                psi2_group(ch0, GROUPS[t])
                ch0 += GROUPS[t]

            # the last chunk as an exp+accumulate single on ACT: nothing
            # left for DVE to drain after the final exp
            for ch in range(ch0, 17):
                p2 = ppool.tile([128, 4 * NLOC], f32, tag="p2")
                nc.tensor.matmul(
                    p2[:, 0:NLOC],
                    lhsT=zl_sb[:, ch * 128:(ch + 1) * 128],
                    rhs=nprep[0:65, :],
                    start=True, stop=True)
                scr = spool.tile([128, 4 * NLOC], f32, tag="p2scr")
                nc.scalar.activation(scr[:, 0:NLOC], p2[:, 0:NLOC], AF.Exp,
                                     accum_out=stats[:, ch:ch + 1])

            nc.sync.dma_start(out=st_o[:, :], in_=stats[:, :])

    nc.compile()
    return nc


def _get_compiled():
    global _compiled
    if _compiled is None:
        _compiled = _build_bass()
    return _compiled


def _np_softplus(x):
    return np.logaddexp(x, 0.0)


def kernel(y, q_mu, q_log_sigma, z, noise_raw, alpha, variance, _trace=False):
    from concourse.bass_utils import run_bass_kernel_spmd

    nc = _get_compiled()

    f8 = np.float64
    z64 = z.astype(f8)
    al = alpha.astype(f8)
    var = f8(variance[0])
    logvar = np.log(var)

    # z-side stationary blocks (host-built, replicated to all cores).
    # psi2 is symmetric in (i, j): ship only the 2080 upper-tri pairs.
    iu, ju = np.triu_indices(M)                             # (2080,)
    npairs = iu.shape[0]
    Su = z64[iu] + z64[ju]                                  # (2080, q)
    sqz = (z64[:, None, :] - z64[None, :, :]) ** 2          # (m, m, q)
    s1 = 0.25 * (sqz @ al)                                  # (m, m)
    zl = np.zeros((65, 17 * 128), np.float32)
    zl[0:16, :npairs] = Su.T
    zl[16:32, :npairs] = (-0.25 * Su * Su).T
    zl[32:48, :npairs] = -1.0
    zl[48:64, :npairs] = -0.5
    zl[64, :npairs] = -s1[iu, ju] + 4.0 * logvar

    zt = z64.T                                              # (q, m)
    zs1x = np.zeros((97, M), np.float32)
    zs1x[65:81] = zt
    zs1x[81:97] = -0.5 * zt * zt

    qmu32 = q_mu.astype(np.float32)
    qls32 = q_log_sigma.astype(np.float32)
    y32 = y.astype(np.float32)

    in_maps = []
    for i in range(NCORES):
        r = i * NLOC
        acqin = np.zeros((128, ACQ_W), np.float32)
        acqin[:, C_QM:C_QM + 16] = qmu32[r:r + 128]
        acqin[:, C_QM + 16:C_QM + 32] = qmu32[r + 128:r + 256]
        acqin[:, C_QLS:C_QLS + 16] = qls32[r:r + 128]
        acqin[:, C_QLS + 16:C_QLS + 32] = qls32[r + 128:r + 256]
        acqin[:, C_AL:C_AL + 16] = alpha.reshape(1, Q)
        acqin[:, C_AL + 16:C_AL + 32] = alpha.reshape(1, Q)
        acqin[:, C_CONST] = 2.0 * logvar
        ybig = np.empty((128, 2 * NLOC), np.float32)
        ybig[:, 0:NLOC] = y32[r:r + 128]
        ybig[:, NLOC:2 * NLOC] = y32[r + 128:r + 256]
        in_maps.append({"acqin": acqin, "zl": zl, "ybig": ybig, "zs1x": zs1x})

    br = run_bass_kernel_spmd(nc, in_maps, list(range(NCORES)), trace=_trace)
    res = br.results

    stats = np.zeros((128, 25), f8)
    A = np.zeros((M, D), f8)
    lnsig = 0.0
    for r in res:
        st = r["out_stats"].astype(f8)
        lnsig += np.sum(np.log(st[:, 19:21]))
        stats += st
        A += r["out_A"].astype(f8)

    flat = stats[:, 0:17].T.reshape(17 * 128)
    psi2 = np.empty((M, M), f8)
    psi2[iu, ju] = flat[:npairs]
    psi2[ju, iu] = flat[:npairs]
    col = stats.sum(axis=0)
    musq = col[23]
    ssq = col[24]
    tr_yy = float(np.sum(y.astype(f8) ** 2))

    kl_sum = -lnsig + 0.5 * (ssq + musq) - 0.5 * N * Q
    kl_term = kl_sum / (N * D)

    # small m x m algebra on host
    k_mm = var * np.exp(-0.5 * (sqz @ al))                  # (m, m)
    noise_var = _np_softplus(f8(noise_raw[0]))
    beta = 1.0 / noise_var
    psi0 = N * var

    cov1 = beta * psi2 + k_mm
    B = np.linalg.solve(cov1, A)
    tr_yWy = beta * tr_yy - np.sum(A * B)

    F = 0.5 * N * np.log(beta)
    F += 0.5 * np.linalg.slogdet(k_mm)[1]
    F -= 0.5 * N * np.log(np.pi)
    F -= 0.5 * np.linalg.slogdet(cov1)[1]
    F -= 0.5 * beta * psi0
    F += 0.5 * np.trace(np.linalg.solve(k_mm, psi2))
    F = (F * D - 0.5 * tr_yWy) / (N * D)

    out = F - kl_term
    result = np.asarray(out, dtype=np.float32)
    if _trace:
        return result, br
    return result


# revision 45
# speedup vs baseline: 1.0212x; 1.0212x over previous
"""Bayesian GPLVM collapsed-ELBO kernel for Trainium2 (8 NeuronCores).

Sharding: data-parallel over n (2048 rows -> 256 per core). Each core
computes its partial psi2 = sum_n exp(log_psi2_n) over the 2080
upper-triangle (i,j) pairs, partial A = psi1^T y (64x256), and partial
KL statistics. The host sums the 8 partials and does the small m x m
linear algebra (solves / slogdets) to produce the scalar ELBO;
tr(y y^T) is an input-only reduction and is also done on the host.

Device flow per core (n_loc = 256 as two fused 128-row chunks):
  - Inputs ride the sync-engine DMA ring; the psi2 z-block zl (the
    per-partition-bytes hog) is split in three pieces with zs1/y
    slotted between, so no matmul group is ever DMA-gated.
  - Prep is fused across both chunks with chunk-strided [p, 2, w]
    views. The per-n reductions that the reference folds into scalar
    exponent rows are instead EXTENDED CONTRACTIONS: qmu^2*w2 and
    ln d2 become matmul rows paired with constant z-side rows (-1 and
    -0.5), so no reduce gates the PE transpose; psi1's h1 scalar is
    injected as a per-partition bias on its exp instead of a matmul
    row. The transpose fires right after four elementwise multiplies.
  - nprep (98 x 256 q-major, one transpose+cast per chunk): rows 0:65
    serve psi2 (zl rows [Su; -0.25 Su^2; -1; -0.5; 4*logvar - s1]),
    rows 0:97 serve psi1 (zs1x zero below row 65, then z, -0.5 z^2).
  - psi2 runs as matmul groups of [2,4,4,4,1] chunks, Exp on Scalar,
    n-sum reduces on Vector with 3-deep PSUM pipelining; the last two
    chunks are exp+accumulate singles on Scalar so the Vector reduce
    stream and the exp stream finish together. A = psi1^T y uses its
    own PSUM bank and is DMA'd out mid-kernel.
  - All activations use only {Exp, Ln}: the activation-table universe
    handed to the table-load pass is filtered (indices preserved) so
    both are served by the one table holding exp AND ln -> a single
    ACT_TABLE_LOAD that overlaps the input DMAs.
"""

import numpy as np

N, D, Q, M = 2048, 256, 16, 64
NCORES = 8
NLOC = N // NCORES          # 256

# acqin column layout
C_QM = 0        # [0:32)    qmu  (chunk0 | chunk1)
C_QLS = 32      # [32:64)   q_log_sigma
C_AL = 64      # [64:96)   alpha tiled twice
C_CONST = 96    # [96:100)  [2*logvar, 4*logvar, 0, 0]
ACQ_W = 104

_compiled = None


def _patch_act_tables():
    """Filter the activation-table universe handed to the table-load
    pass so `natural_log_exp_and_others` is the only table providing
    Exp/Ln. Table indices (act_func_set_id) are preserved, so the
    emitted loads still point at the real act_info.json entry; the
    whole kernel then needs a single ACT_TABLE_LOAD."""
    import concourse.bacc as bacc_mod
    import concourse.mybir as mybir
    from concourse.hw_specs import get_activation_tables

    def patched(arch):
        real = get_activation_tables(arch)
        target = None
        for name, funcs in real.items():
            if (mybir.ActivationFunctionType.Exp in funcs
                    and mybir.ActivationFunctionType.Ln in funcs):
                target = name
                break
        if target is None:
            return real
        strip = {mybir.ActivationFunctionType.Exp,
                 mybir.ActivationFunctionType.Ln}
        return {
            name: (set(funcs) if name == target else set(funcs) - strip)
            for name, funcs in real.items()
        }

    bacc_mod.get_activation_tables = patched


def _build_bass():
    import concourse.bacc as bacc
    import concourse.mybir as mybir
    from concourse import masks
    from concourse.tile import TileContext

    _patch_act_tables()

    f32 = mybir.dt.float32
    f32r = mybir.dt.float32r
    bf16 = mybir.dt.bfloat16
    AF = mybir.ActivationFunctionType
    OP = mybir.AluOpType
    AX = mybir.AxisListType

    nc = bacc.Bacc("TRN2", target_bir_lowering=False, num_swdge_queues=2)

    acqin_d = nc.declare_dram_parameter("acqin", [128, ACQ_W], f32, isOutput=False)
    zl_d = nc.declare_dram_parameter("zl", [65, 17 * 128], f32r, isOutput=False)
    y_d = nc.declare_dram_parameter("ybig", [128, 2 * NLOC], f32r, isOutput=False)
    zs1_d = nc.declare_dram_parameter("zs1x", [97, M], f32r, isOutput=False)
    a_o = nc.declare_dram_parameter("out_A", [M, D], f32, isOutput=True)
    st_o = nc.declare_dram_parameter("out_stats", [128, 25], f32, isOutput=True)

    with TileContext(nc) as tc:
        with (
            tc.tile_pool(name="const", bufs=1) as cpool,
            tc.tile_pool(name="big", bufs=1) as bigpool,
            tc.tile_pool(name="scr", bufs=3) as spool,
            tc.tile_pool(name="psum", bufs=3, space="PSUM") as ppool,
            tc.tile_pool(name="psums", bufs=1, space="PSUM") as ppools,
        ):
            # inputs, all on the sync (SP) ring. The 34-partition zl
            # transfer (the per-partition-bytes hog) is split in three so
            # the first psi2 groups are never DMA-gated; y rides between
            # the second and third zl pieces (needed by the A matmuls).
            acqin = cpool.tile([128, ACQ_W], f32)
            nc.sync.dma_start(out=acqin[:, :], in_=acqin_d[:, :])
            zl_sb = bigpool.tile([65, 17 * 128], f32r)
            zc1, zc2 = 2 * 128, 10 * 128
            nc.sync.dma_start(out=zl_sb[:, :zc1], in_=zl_d[:, :zc1])
            zs1_sb = cpool.tile([97, M], f32r)
            nc.sync.dma_start(out=zs1_sb[:, :], in_=zs1_d[:, :])
            nc.sync.dma_start(out=zl_sb[:, zc1:zc2], in_=zl_d[:, zc1:zc2])
            ybig = bigpool.tile([128, 2 * NLOC], f32r)
            nc.sync.dma_start(out=ybig[:, :], in_=y_d[:, :])
            nc.sync.dma_start(out=zl_sb[:, zc2:], in_=zl_d[:, zc2:])

            ident = cpool.tile([128, 128], f32)
            masks.make_identity(nc, ident[:])

            qm = acqin[:, C_QM:C_QM + 32]
            qls = acqin[:, C_QLS:C_QLS + 32]
            al2 = acqin[:, C_AL:C_AL + 32]
            c2lv = acqin[:, C_CONST:C_CONST + 1]
            zs1x = zs1_sb[:, :]

            stats = bigpool.tile([128, 25], f32)
            nprep = bigpool.tile([98, 2 * 128], f32r)
            P = bigpool.tile([128, 196], f32)
            S = bigpool.tile([128, 96], f32)      # [d1(32) | d2(32) | qsig(32)]
            L = bigpool.tile([128, 96], f32)      # ln of S
            M1 = bigpool.tile([128, 32], f32)
            R = bigpool.tile([128, 64], f32)
            scr2 = bigpool.tile([128, 64], f32)
            cols = bigpool.tile([128, 8], f32)
            qmsq = bigpool.tile([128, 32], f32)

            qsig = S[:, 64:96]
            d1 = S[:, 0:32]
            d2 = S[:, 32:64]

            # q_sigma = softplus(qls) = ln(1 + exp(qls)), both chunks
            nc.vector.tensor_mul(qmsq[:, :], qm, qm)
            nc.scalar.activation(M1[:, :], qls, AF.Exp)
            nc.scalar.activation(qsig, M1[:, :], AF.Ln, bias=1.0)
            # d1 = alpha*qsig + 1 ; d2 = 2*d1 - 1 = 2*(alpha*qsig) + 1
            nc.vector.tensor_mul(M1[:, :], qsig, al2)
            nc.vector.tensor_scalar_add(d1, M1[:, :], 1.0)
            nc.vector.tensor_scalar(out=d2, in0=M1[:, :], scalar1=2.0,
                                    scalar2=1.0, op0=OP.mult, op1=OP.add)
            # Ln over [d1|qsig] feeds the h1 bias and the KL ln-sigma
            nc.scalar.activation(L[:, 0:32], d1, AF.Ln)
            nc.scalar.activation(L[:, 32:64], qsig, AF.Ln)
            # SR cols: [sum2_c0, sum2_c1, lnsig_c0, lnsig_c1]
            nc.vector.tensor_reduce(
                stats[:, 17:21], L[:, 0:64].rearrange("p (a b) -> p a b", b=16),
                axis=AX.X, op=OP.add)

            # Both chunks at once via chunk-strided [p, 2, w] views of P.
            # Per-chunk columns (= nprep rows after the transpose):
            # 0:16 qmu*w2, 16:32 w2, 32:48 qmu^2*w2, 48:64 ln d2,
            # 64 one, 65:81 qmu*w1, 81:97 w1, 97 pad.
            # psi2 contracts rows 0:65 against zl rows
            # [Su; -0.25*Su^2; -1; -0.5; 4*logvar - s1]; the qmu^2*w2 and
            # ln d2 rows replace the old per-n "g" scalar, so no reduce
            # gates the transpose. psi1 contracts rows 0:97 against zs1x
            # (zeros below 65, then z, -0.5 z^2), with h1 injected later
            # as a per-partition bias on the psi1 exp.
            Pv = P[:, :].rearrange("p (c s) -> p c s", c=2)
            qmv = qm.rearrange("p (c s) -> p c s", s=16)
            qmsqv = qmsq[:, :].rearrange("p (c s) -> p c s", s=16)
            alv = al2.rearrange("p (c s) -> p c s", s=16)
            nc.vector.memset(Pv[:, :, 64:65], 1.0)
            nc.vector.reciprocal(R[:, :], S[:, 0:64])
            Rv1 = R[:, 0:32].rearrange("p (c s) -> p c s", s=16)
            Rv2 = R[:, 32:64].rearrange("p (c s) -> p c s", s=16)
            nc.vector.tensor_mul(Pv[:, :, 16:32], Rv2, alv)
            nc.vector.tensor_mul(Pv[:, :, 81:97], Rv1, alv)
            nc.vector.tensor_mul(Pv[:, :, 0:16], Pv[:, :, 16:32], qmv)
            nc.vector.tensor_mul(Pv[:, :, 32:48], Pv[:, :, 0:16], qmv)
            nc.vector.tensor_mul(Pv[:, :, 65:81], Pv[:, :, 81:97], qmv)
            nc.scalar.activation(Pv[:, :, 48:64], d2, AF.Ln)

            ptp = ppools.tile([98, 256], f32, tag="ptp")
            for c in range(2):
                nc.tensor.transpose(ptp[:, 128 * c:128 * (c + 1)],
                                    P[:, 98 * c:98 * (c + 1)], ident[:, :])
            nc.vector.tensor_copy(nprep[:, :], ptp[:, :])

            # h1 = 2*logvar - 0.5*(rt1 + sum2), used as psi1 exp bias
            nc.vector.tensor_mul(
                scr2[:, 0:32].rearrange("p (c s) -> p c s", s=16),
                Pv[:, :, 81:97], qmsqv)
            nc.vector.tensor_reduce(
                cols[:, 0:2], scr2[:, 0:32].rearrange("p (a b) -> p a b", b=16),
                axis=AX.X, op=OP.add)
            nc.vector.tensor_scalar(
                out=cols[:, 4:6], in0=stats[:, 17:19], scalar1=-0.5,
                scalar2=c2lv, op0=OP.mult, op1=OP.add)
            nc.vector.scalar_tensor_tensor(
                out=cols[:, 6:8], in0=cols[:, 0:2], scalar=-0.5,
                in1=cols[:, 4:6], op0=OP.mult, op1=OP.add)

            # KL statistics (tr(y y^T) is an input-only reduction and is
            # done on the host)
            nc.vector.tensor_mul(scr2[:, 0:32], qsig, qsig)
            nc.vector.tensor_reduce(
                stats[:, 23:24], qmsq[:, :].rearrange("p (a b) -> p a b", b=32),
                axis=AX.X, op=OP.add)
            nc.vector.tensor_reduce(
                stats[:, 24:25], scr2[:, 0:32].rearrange("p (a b) -> p a b", b=32),
                axis=AX.X, op=OP.add)

            # psi2: 17 ij-chunks in groups (first group small so the ACT
            # exp stream starts as early as possible); n-sums on DVE
            GROUPS = (2, 4, 4, 4, 1)

            def psi2_group(ch0, nch):
                w = nch * NLOC
                p2 = ppool.tile([128, 4 * NLOC], f32, tag="p2")
                for j in range(nch):
                    ch = ch0 + j
                    nc.tensor.matmul(
                        p2[:, j * NLOC:(j + 1) * NLOC],
                        lhsT=zl_sb[:, ch * 128:(ch + 1) * 128],
                        rhs=nprep[0:65, :],
                        start=True, stop=True)
                scr = spool.tile([128, 4 * NLOC], f32, tag="p2scr")
                nc.scalar.activation(scr[:, :w], p2[:, :w], AF.Exp)
                nc.vector.tensor_reduce(
                    stats[:, ch0:ch0 + nch],
                    scr[:, :w].rearrange("p (a b) -> p a b", b=NLOC),
                    axis=AX.X, op=OP.add)

            psi2_group(0, GROUPS[0])

            # psi1 exponent for both chunks (borrows one rotation slot of
            # the psi2 PSUM pool, released by its exp)
            e1 = ppool.tile([128, 4 * NLOC], f32, tag="p2")
            for c in range(2):
                nc.tensor.matmul(e1[:, M * c:M * (c + 1)],
                                 lhsT=nprep[0:97, 128 * c:128 * (c + 1)],
                                 rhs=zs1x,
                                 start=True, stop=True)
            psi1c = bigpool.tile([128, 2 * M], f32r)
            for c in range(2):
                nc.scalar.activation(psi1c[:, M * c:M * (c + 1)],
                                     e1[:, M * c:M * (c + 1)], AF.Exp,
                                     bias=cols[:, 6 + c:7 + c])

            psi2_group(GROUPS[0], GROUPS[1])

            # A = psi1^T y early (own PSUM bank): its copy slots into a
            # DVE gap and the DMA completes long before the tail
            apsum = ppools.tile([M, D], f32, tag="aps")
            for c in range(2):
                nc.tensor.matmul(apsum[:, :],
                                 lhsT=psi1c[:, M * c:M * (c + 1)],
                                 rhs=ybig[:, NLOC * c:NLOC * (c + 1)],
                                 start=(c == 0), stop=(c == 1))
            a_sb = bigpool.tile([M, D], f32)
            nc.vector.tensor_copy(a_sb[:, :], apsum[:, :])
            nc.sync.dma_start(out=a_o[:, :], in_=a_sb[:, :])

            ch0 = GROUPS[0] + GROUPS[1]
            for t in range(2, len(GROUPS)):
                psi2_group(ch0, GROUPS[t])
                ch0 += GROUPS[t]

            # the last chunk as an exp+accumulate single on ACT: nothing
            # left for DVE to drain after the final exp
            for ch in range(ch0, 17):
                p2 = ppool.tile([128, 4 * NLOC], f32, tag="p2")
                nc.tensor.matmul(
                    p2[:, 0:NLOC],
                    lhsT=zl_sb[:, ch * 128:(ch + 1) * 128],
                    rhs=nprep[0:65, :],
                    start=True, stop=True)
                scr = spool.tile([128, 4 * NLOC], f32, tag="p2scr")
                nc.scalar.activation(scr[:, 0:NLOC], p2[:, 0:NLOC], AF.Exp,
                                     accum_out=stats[:, ch:ch + 1])

            nc.sync.dma_start(out=st_o[:, :], in_=stats[:, :])

    nc.compile()
    return nc


def _get_compiled():
    global _compiled
    if _compiled is None:
        _compiled = _build_bass()
    return _compiled


def _np_softplus(x):
    return np.logaddexp(x, 0.0)


def kernel(y, q_mu, q_log_sigma, z, noise_raw, alpha, variance, _trace=False):
    from concourse.bass_utils import run_bass_kernel_spmd

    nc = _get_compiled()

    f8 = np.float64
    z64 = z.astype(f8)
    al = alpha.astype(f8)
    var = f8(variance[0])
    logvar = np.log(var)

    # z-side stationary blocks (host-built, replicated to all cores).
    # psi2 is symmetric in (i, j): ship only the 2080 upper-tri pairs.
    iu, ju = np.triu_indices(M)                             # (2080,)
    npairs = iu.shape[0]
    Su = z64[iu] + z64[ju]                                  # (2080, q)
    sqz = (z64[:, None, :] - z64[None, :, :]) ** 2          # (m, m, q)
    s1 = 0.25 * (sqz @ al)                                  # (m, m)
    zl = np.zeros((65, 17 * 128), np.float32)
    zl[0:16, :npairs] = Su.T
    zl[16:32, :npairs] = (-0.25 * Su * Su).T
    zl[32:48, :npairs] = -1.0
    zl[48:64, :npairs] = -0.5
    zl[64, :npairs] = -s1[iu, ju] + 4.0 * logvar

    zt = z64.T                                              # (q, m)
    zs1x = np.zeros((97, M), np.float32)
    zs1x[65:81] = zt
    zs1x[81:97] = -0.5 * zt * zt

    qmu32 = q_mu.astype(np.float32)
    qls32 = q_log_sigma.astype(np.float32)
    y32 = y.astype(np.float32)

    in_maps = []
    for i in range(NCORES):
        r = i * NLOC
        acqin = np.zeros((128, ACQ_W), np.float32)
        acqin[:, C_QM:C_QM + 16] = qmu32[r:r + 128]
        acqin[:, C_QM + 16:C_QM + 32] = qmu32[r + 128:r + 256]
        acqin[:, C_QLS:C_QLS + 16] = qls32[r:r + 128]
        acqin[:, C_QLS + 16:C_QLS + 32] = qls32[r + 128:r + 256]
        acqin[:, C_AL:C_AL + 16] = alpha.reshape(1, Q)
        acqin[:, C_AL + 16:C_AL + 32] = alpha.reshape(1, Q)
        acqin[:, C_CONST] = 2.0 * logvar
        ybig = np.empty((128, 2 * NLOC), np.float32)
        ybig[:, 0:NLOC] = y32[r:r + 128]
        ybig[:, NLOC:2 * NLOC] = y32[r + 128:r + 256]
        in_maps.append({"acqin": acqin, "zl": zl, "ybig": ybig, "zs1x": zs1x})

    br = run_bass_kernel_spmd(nc, in_maps, list(range(NCORES)), trace=_trace)
    res = br.results

    stats = np.zeros((128, 25), f8)
    A = np.zeros((M, D), f8)
    for r in res:
        stats += r["out_stats"].astype(f8)
        A += r["out_A"].astype(f8)

    flat = stats[:, 0:17].T.reshape(17 * 128)
    psi2 = np.empty((M, M), f8)
    psi2[iu, ju] = flat[:npairs]
    psi2[ju, iu] = flat[:npairs]
    col = stats.sum(axis=0)
    lnsig = col[19] + col[20]
    musq = col[23]
    ssq = col[24]
    tr_yy = float(np.sum(y.astype(f8) ** 2))

    kl_sum = -lnsig + 0.5 * (ssq + musq) - 0.5 * N * Q
    kl_term = kl_sum / (N * D)

    # small m x m algebra on host
    k_mm = var * np.exp(-0.5 * (sqz @ al))                  # (m, m)
    noise_var = _np_softplus(f8(noise_raw[0]))
    beta = 1.0 / noise_var
    psi0 = N * var

    cov1 = beta * psi2 + k_mm
    B = np.linalg.solve(cov1, A)
    tr_yWy = beta * tr_yy - np.sum(A * B)

    F = 0.5 * N * np.log(beta)
    F += 0.5 * np.linalg.slogdet(k_mm)[1]
    F -= 0.5 * N * np.log(np.pi)
    F -= 0.5 * np.linalg.slogdet(cov1)[1]
    F -= 0.5 * beta * psi0
    F += 0.5 * np.trace(np.linalg.solve(k_mm, psi2))
    F = (F * D - 0.5 * tr_yWy) / (N * D)

    out = F - kl_term
    result = np.asarray(out, dtype=np.float32)
    if _trace:
        return result, br
    return result
